# revision 36
# baseline (speedup 1.0000x reference)
"""Multi-head causal attention (B=2, T=2048, C=1024, H=16) on 8 trn2 NeuronCores.

Sharding: 2 heads per core (tensor-parallel over heads), both batch elements
on every core. Per core:
  1. qkv projection for its 2 heads in four 512-token chunks, interleaved with
     the attention chunks that consume them (q^T,k^T in [d, t] layout, v in
     natural [t, d] layout; fp16 matmuls, fp32 PSUM accumulation). Input x^T
     is DMA'd per (batch, chunk) so the first matmuls start ~6us in.
  2. Flash-style causal attention in the S^T = k q^T layout: exp on ScalarE
     straight out of PSUM (logits are O(1) by construction). The two heads'
     S matmuls run concurrently in disjoint PE row groups (contract dim 64
     each). V tiles are laid out [V0 | ones | ones | V1] so each head's P@V
     matmul emits its softmax row-sums pre-broadcast on the opposite
     64-partition half; reciprocal + normalize then run partition-aligned
     on VectorE (custom DVE ops only work at base partition 0).
  3. One AllToAll at a drained point exchanges both batches' attention
     outputs (dest chunk p = [128 chans, 256 b0 rows | 256 b1 rows]); each
     core then projects its 512 rows. Mid-compute collectives measured
     16-44us for the same payload (fabric contention), so the exchange is
     deliberately not overlapped with compute; dependency-pinned dummy
     matmuls keep the PE clock warm through it. A tiny warm-up AllToAll
     issued early hides the one-time channel-setup barrier (~50us).
Host side shards/transposes/casts inputs (fp16) and reassembles the output.
"""

import sys

import numpy as np

if "/opt/trn_rl_repo" not in sys.path:
    sys.path.insert(0, "/opt/trn_rl_repo")

B, T, C, H, D = 2, 2048, 1024, 16, 64
NCORES = 8
HPC = H // NCORES          # heads per core = 2
CW = HPC * D               # per-core channel width = 128
KT = C // 128              # k tiles = 8
TT = T // 128              # t tiles = 16
TJ = 4                     # token chunks per batch (512 tokens each)
HROWS = 256                # rows per (core, batch) after exchange
SCALE = 1.0 / float(np.sqrt(C))

_CACHE = {}
LAST_EXEC_NS = None


def _build_nc():
    import concourse.mybir as mybir
    import concourse.tile as tile
    from concourse import bacc
    from concourse.masks import make_identity, make_upper_triangular

    f32 = mybir.dt.float32
    f16 = mybir.dt.float16

    nc = bacc.Bacc("TRN2", target_bir_lowering=False, debug=False,
                   num_devices=NCORES)

    # xT chunk (b, j): [128, KT*512], k-tile a at cols [a*512:(a+1)*512],
    # covering tokens [512j, 512(j+1)) of batch b.
    xT = nc.dram_tensor("xT", [B * TJ, 128, KT * 512], f16, kind="ExternalInput")
    wq = nc.dram_tensor("wq", [128, KT * CW], f16, kind="ExternalInput")
    wk = nc.dram_tensor("wk", [128, KT * CW], f16, kind="ExternalInput")
    wv = nc.dram_tensor("wv", [128, KT * CW], f16, kind="ExternalInput")
    wp = nc.dram_tensor("wp", [128, KT * C], f16, kind="ExternalInput")
    bq = nc.dram_tensor("bq", [CW, 1], f32, kind="ExternalInput")
    bk = nc.dram_tensor("bk", [CW, 1], f32, kind="ExternalInput")
    bv = nc.dram_tensor("bv", [CW, 1], f32, kind="ExternalInput")
    bp = nc.dram_tensor("bp", [1, C], f32, kind="ExternalInput")
    y = nc.dram_tensor("y", [2 * HROWS, C], f32, kind="ExternalOutput")

    with tile.TileContext(nc) as tc:
        with (
            tc.tile_pool(name="const", bufs=1) as const,
            tc.tile_pool(name="dram", bufs=1, space="DRAM") as dram,
            tc.tile_pool(name="xtp", bufs=1) as xtp,
            tc.tile_pool(name="wqkv", bufs=1) as wqkvp,
            tc.tile_pool(name="qkv", bufs=1) as qkvp,
            tc.tile_pool(name="pt", bufs=5) as ptp,
            tc.tile_pool(name="otp", bufs=1) as otp,
            tc.tile_pool(name="sm", bufs=2) as smp,
            tc.tile_pool(name="proj", bufs=1) as projp,
            tc.tile_pool(name="ysb", bufs=2) as ysbp,
        ):
            # ---- collective warm-up (channel init overlaps compute) ----
            warm_i = dram.tile([8, 16], f32, name="warm_i")
            warm_o = dram.tile([8, 16], f32, name="warm_o")
            wtile = const.tile([8, 16], f32, name="wtile")
            nc.vector.memset(wtile[:], 0.0)
            nc.sync.dma_start(warm_i[:], wtile[:])
            nc.gpsimd.collective_compute(
                "AllToAll", mybir.AluOpType.bypass,
                replica_groups=[list(range(NCORES))],
                ins=[warm_i[:].opt()], outs=[warm_o[:].opt()],
            )

            # ---- critical-path DMAs first: wq + first x chunk ----
            wq_sb = wqkvp.tile([128, KT * CW], f16, name="wq_sb")
            wk_sb = wqkvp.tile([128, KT * CW], f16, name="wk_sb")
            wv_sb = wqkvp.tile([128, KT * CW], f16, name="wv_sb")
            xt_tiles = {}

            def dma_xt(b, j):
                xt = xtp.tile([128, KT * 512], f16, name=f"xt{b}_{j}")
                nc.sync.dma_start(xt[:], xT[TJ * b + j])
                xt_tiles[b, j] = xt

            dma_xt(0, 0)
            nc.sync.dma_start(wq_sb[:], wq[:])
            nc.sync.dma_start(wk_sb[:], wk[:])
            dma_xt(0, 1)
            nc.sync.dma_start(wv_sb[:], wv[:])

            # ---- constants ----
            trimask = const.tile([128, 128], f16, name="trimask")
            make_upper_triangular(nc, trimask[:], val=1.0, diag=True)
            ident = const.tile([128, 128], f16, name="ident")
            make_identity(nc, ident[:])

            bq_t = const.tile([CW, 1], f32, name="bq_t")
            bk_t = const.tile([CW, 1], f32, name="bk_t")
            nc.sync.dma_start(bq_t[:], bq[:])
            nc.sync.dma_start(bk_t[:], bk[:])
            bv_t = const.tile([CW, 1], f32, name="bv_t")
            nc.sync.dma_start(bv_t[:], bv[:])
            bp_row = const.tile([1, C], f32, name="bp_row")
            nc.sync.dma_start(bp_row[:], bp[:])
            bpb = const.tile([128, C], f32, name="bpb")
            nc.gpsimd.partition_broadcast(bpb[:], bp_row[:])

            # ---- remaining x chunks, then wp ----
            dma_xt(0, 2)
            dma_xt(0, 3)
            for j in range(TJ):
                dma_xt(1, j)
            wp_sb = projp.tile([128, KT * C], f16, name="wp_sb")
            nc.sync.dma_start(wp_sb[:], wp[:])

            # ---- PSUM pools (shared across qkv / attention / proj) ----
            psmm_pool = tc.tile_pool(name="psmm", bufs=2, space="PSUM")
            psmm = psmm_pool.__enter__()
            attn_psum_s = tc.tile_pool(name="ps_s", bufs=2, space="PSUM")
            ps_s = attn_psum_s.__enter__()
            attn_psum_o = tc.tile_pool(name="ps_o", bufs=1, space="PSUM")
            ps_o = attn_psum_o.__enter__()

            # PE warm-up: dummy matmuls on trimask (SBUF-resident, no DMA
            # dependency) keep the HAM activity monitor at full clock while
            # x streams in.
            warm_ps = psmm.tile([128, 128], f32, name="warm_ps", tag="mm")
            for _ in range(64):
                nc.tensor.matmul(
                    warm_ps[:], trimask[:], trimask[:],
                    start=True, stop=True,
                )
            nc.vector.memset(warm_ps[:, 0:2], 0.0)

            qT_sb, kT_sb, v_sb, ot_sb = {}, {}, {}, {}
            # single exchange for both batches at a quiet point: dest chunk p
            # = [128 chans, 256 b0 rows | 256 b1 rows].  Mid-compute
            # collectives measured wildly variable (16-44us for 512KB) from
            # fabric contention; one drained-point exchange is deterministic.
            a2a_in = dram.tile([NCORES, 128, 2 * HROWS], f16, name="a2a_in")
            a2a_out = dram.tile([NCORES, 128, 2 * HROWS], f16, name="a2a_out")
            for b in range(B):
                qT_sb[b] = qkvp.tile([128, T], f16, name=f"qT{b}")
                kT_sb[b] = qkvp.tile([128, T], f16, name=f"kT{b}")
                ot_sb[b] = otp.tile([128, T], f16, name=f"ot{b}")
                v_sb[b] = []

            def emit_qkv_chunk(b, j):
                """q/k/v projection for tokens [512j, 512(j+1)) of batch b."""
                xt = xt_tiles[b, j]
                vT_j = qkvp.tile([128, 512], f16, name=f"vT{b}_{j}")
                for dst, w_sb, bias in (
                    (qT_sb[b][:, 512 * j: 512 * (j + 1)], wq_sb, bq_t),
                    (kT_sb[b][:, 512 * j: 512 * (j + 1)], wk_sb, bk_t),
                    (vT_j[:], wv_sb, bv_t),
                ):
                    ps = psmm.tile([128, 512], f32, name="ps_mm", tag="mm")
                    for a in range(KT):
                        nc.tensor.matmul(
                            ps[:],
                            w_sb[:, CW * a: CW * (a + 1)],
                            xt[:, 512 * a: 512 * (a + 1)],
                            start=(a == 0), stop=(a == KT - 1),
                        )
                    nc.vector.tensor_scalar_add(dst, ps[:], bias[:])
                for m in range(4):
                    # [V0 | ones | ones | V1]: each head's PV lhsT is a
                    # [128, 128] slice whose 64 ones-columns produce the
                    # softmax row-sums broadcast on the other partition half
                    vt = qkvp.tile([128, 4 * D], f16, name=f"v{b}_{4 * j + m}")
                    tps = psmm.tile([128, 128], f16, name="ps_tr", tag="mm")
                    nc.tensor.transpose(
                        tps[:], vT_j[:, 128 * m: 128 * (m + 1)], ident[:]
                    )
                    nc.vector.tensor_copy(vt[:, 0:D], tps[:, 0:D])
                    nc.vector.memset(vt[:, D: 3 * D], 1.0)
                    nc.vector.tensor_copy(vt[:, 3 * D: 4 * D], tps[:, D: 2 * D])
                    v_sb[b].append(vt)

            def emit_attn_chunk(b, j):
                """Causal attention for query tokens [512j, 512(j+1))."""
                ot = ot_sb[b]
                o_ps = [
                    ps_o.tile([128, 512], f32, name=f"o{h}", tag=f"o{h}")
                    for h in range(2)
                ]
                ilast = 4 * (j + 1) - 1
                for i in range(4 * (j + 1)):
                    off = max(0, 128 * i - 512 * j)
                    w = 512 - off
                    # one [128,1024] tile, head h in bank h
                    s_ps = ps_s.tile([128, 1024], f32, name="s_ps", tag="s")
                    pt = ptp.tile([128, 1024], f16, name="pt", tag="pt")
                    for h in range(2):
                        nc.tensor.matmul(
                            s_ps[:, 512 * h + off: 512 * (h + 1)],
                            kT_sb[b][64 * h: 64 * h + 64,
                                     128 * i: 128 * (i + 1)],
                            qT_sb[b][64 * h: 64 * h + 64,
                                     512 * j + off: 512 * (j + 1)],
                            start=True, stop=True,
                        )
                    nc.scalar.activation(
                        pt[:].rearrange("p (g w) -> p g w", g=2)[:, :, off:512],
                        s_ps[:].rearrange("p (g w) -> p g w", g=2)[:, :, off:512],
                        mybir.ActivationFunctionType.Exp,
                        scale=SCALE,
                    )
                    if 4 * j <= i:
                        for h in range(2):
                            nc.vector.tensor_tensor(
                                pt[:, 512 * h + off: 512 * h + off + 128],
                                pt[:, 512 * h + off: 512 * h + off + 128],
                                trimask[:],
                                op=mybir.AluOpType.mult,
                            )
                    for h in range(2):
                        nc.tensor.matmul(
                            o_ps[h][:, off:512],
                            v_sb[b][i][:, 2 * D * h: 2 * D * (h + 1)],
                            pt[:, 512 * h + off: 512 * (h + 1)],
                            start=(i == 0), stop=(i == ilast),
                        )
                # o_ps[0] = [O0 ; r0*ones], o_ps[1] = [r1*ones ; O1]: the
                # row-sums arrive pre-broadcast on the opposite partition
                # half, so recip + normalize are partition-aligned on DVE
                # custom DVE ops (recip) only work at base partition 0, so
                # reciprocate there and let tensor_copy do partition shifts
                rr = smp.tile([128, 1024], f32, name="rr", tag="rr")
                nc.vector.tensor_copy(rr[0:64, 0:512], o_ps[0][64:128, :])
                nc.vector.reciprocal_approx_fast(rr[0:64, 0:512], rr[0:64, 0:512])
                nc.vector.tensor_tensor(
                    ot[0:64, 512 * j: 512 * (j + 1)],
                    o_ps[0][0:64, :], rr[0:64, 0:512],
                    op=mybir.AluOpType.mult,
                )
                nc.vector.reciprocal_approx_fast(
                    rr[0:64, 512:1024], o_ps[1][0:64, :]
                )
                nc.vector.tensor_copy(rr[64:128, 512:1024], rr[0:64, 512:1024])
                nc.vector.tensor_tensor(
                    ot[64:128, 512 * j: 512 * (j + 1)],
                    o_ps[1][64:128, :], rr[64:128, 512:1024],
                    op=mybir.AluOpType.mult,
                )
                for p in (2 * j, 2 * j + 1):
                    nc.sync.dma_start(
                        a2a_in[p][:, HROWS * b: HROWS * (b + 1)],
                        ot[:, HROWS * p: HROWS * (p + 1)],
                    )

            for b in range(B):
                for j in range(TJ):
                    emit_qkv_chunk(b, j)
                    emit_attn_chunk(b, j)
            nc.gpsimd.collective_compute(
                "AllToAll", mybir.AluOpType.bypass,
                replica_groups=[list(range(NCORES))],
                ins=[a2a_in[:].opt()],
                outs=[a2a_out[:].opt()],
            )
            yT_sb = []
            for k in range(KT):
                yt = projp.tile([128, 2 * HROWS], f16, name=f"yT{k}")
                eng = nc.sync if k % 2 == 0 else nc.scalar
                eng.dma_start(yt[:], a2a_out[k])
                yT_sb.append(yt)

            # dummy matmuls pinned behind the batch-1 attention output keep
            # the PE clock warm through the exchange; they fill the ~32us
            # collective + load window so the projection streams at full rate
            warm2 = ps_s.tile([128, 1024], f32, name="warm2", tag="s")
            for w in range(90):
                nc.tensor.matmul(
                    warm2[:, 0:512],
                    ot_sb[1][:, 1664:1792], ot_sb[1][:, 1536:2048],
                    start=True, stop=True,
                )
            nc.vector.memset(warm2[:, 0:2], 0.0)

            # ---- output projection: two 128-row tiles per batch ----
            def emit_proj(b):
                for m in range(HROWS // 128):
                    ysb = ysbp.tile([128, C], f32, name="ysb", tag="ysb")
                    for n in range(2):
                        ps = psmm.tile([128, 512], f32, name="ps_y", tag="mm")
                        for k in range(KT):
                            nc.tensor.matmul(
                                ps[:],
                                yT_sb[k][:, HROWS * b + 128 * m:
                                           HROWS * b + 128 * (m + 1)],
                                wp_sb[:, C * k + 512 * n: C * k + 512 * (n + 1)],
                                start=(k == 0), stop=(k == KT - 1),
                            )
                        nc.vector.tensor_tensor(
                            ysb[:, 512 * n: 512 * (n + 1)],
                            ps[:],
                            bpb[:, 512 * n: 512 * (n + 1)],
                            op=mybir.AluOpType.add,
                        )
                    r0 = HROWS * b + 128 * m
                    nc.sync.dma_start(y[r0: r0 + 128, :], ysb[:])

            emit_proj(0)
            emit_proj(1)

            attn_psum_o.__exit__(None, None, None)
            attn_psum_s.__exit__(None, None, None)
            psmm_pool.__exit__(None, None, None)

    nc.compile()
    return nc


def _get_nc():
    if "nc" not in _CACHE:
        _CACHE["nc"] = _build_nc()
    return _CACHE["nc"]


def kernel(x, W_attn, b_attn, W_proj, b_proj, _trace=False):
    global LAST_EXEC_NS
    from concourse.bass_utils import run_bass_kernel_spmd

    x = np.asarray(x, np.float32)
    W_attn = np.asarray(W_attn, np.float32)
    b_attn = np.asarray(b_attn, np.float32)
    W_proj = np.asarray(W_proj, np.float32)
    b_proj = np.asarray(b_proj, np.float32)

    def pmajor(w):  # [C, M] -> [128, KT*M], k-tile a at cols [a*M:(a+1)*M]
        m = w.shape[1]
        return np.ascontiguousarray(
            w.reshape(KT, 128, m).transpose(1, 0, 2).reshape(128, KT * m)
        ).astype(np.float16)

    xT = np.transpose(x, (0, 2, 1))  # [B, C, T]
    # [B*TJ, 128, KT*512]: chunk (b, j) has k-tile a at cols [512a, 512(a+1))
    xT16 = np.ascontiguousarray(
        xT.reshape(B, KT, 128, TJ, 512).transpose(0, 3, 2, 1, 4)
        .reshape(B * TJ, 128, KT * 512)
    ).astype(np.float16)
    wp16 = pmajor(W_proj)
    bp_h = np.ascontiguousarray(b_proj).reshape(1, C)

    in_maps = []
    for c in range(NCORES):
        s = slice(CW * c, CW * (c + 1))
        in_maps.append({
            "xT": xT16,
            "wq": pmajor(W_attn[:, s]),
            "wk": pmajor(W_attn[:, C:][:, s]),
            "wv": pmajor(W_attn[:, 2 * C:][:, s]),
            "wp": wp16,
            "bq": np.ascontiguousarray(b_attn[s]).reshape(CW, 1),
            "bk": np.ascontiguousarray(b_attn[C:][s]).reshape(CW, 1),
            "bv": np.ascontiguousarray(b_attn[2 * C:][s]).reshape(1, CW),
            "bp": bp_h,
        })

    nc = _get_nc()
    res = run_bass_kernel_spmd(nc, in_maps, list(range(NCORES)), trace=_trace)
    LAST_EXEC_NS = res.exec_time_ns

    # core c, batch b: rows [256b, 256b+256) are tokens [256c, 256c+256)
    out = np.empty((B, T, C), np.float32)
    for c in range(NCORES):
        for b in range(B):
            out[b, HROWS * c: HROWS * (c + 1), :] = \
                res.results[c]["y"][HROWS * b: HROWS * (b + 1)]
    return out
